# revision 9
# baseline (speedup 1.0000x reference)
"""Trainium2 Bass kernel for nn_AdaptiveGraphConv (gnn_message_passing).

Data-parallel over batch: B=64 split as 8 batch elements per NeuronCore,
params replicated. No collectives needed.

Per-batch-element device pipeline (all matmuls in bf16, PSUM accum f32):
  1. cast-DMA x_b (64, 7500) f32 -> bf16 into rows 0:64 of `stk`
  2. P = (theta^T phi) @ X          -- 15 matmuls N=500, lhsT = ksymT
  3. M via 125-col chunks accumulated into one (125,125) PSUM tile:
       Mps += X_chunk^T @ P_chunk   -- the 5 diagonal (25,25) blocks sum to
       M[n,m] = sum_{c,t} X[c,t,n] P[c,t,m]  (cross-t blocks are discarded)
     interleaved PE transposes X_chunk^T -> xt16 for step 5
  4. softmax rows of M; adj = sum_k(A+Bparam) + 3*softmax(M)
  5. x_sum chunks = X_chunk^T.T @ blockdiag(adj x5) -> rows 64:128 of `stk`
  6. out = [wr|w1] @ stk, fused BN+ReLU via ScalarE activation, DMA out.
     (x occupies rows 0:64 so every matmul operand has base_partition 0)
"""
import numpy as np
import ml_dtypes

B_, CIN, T_, N_ = 64, 64, 300, 25
COUT, EMB, KV = 128, 32, 3
EPS = 1e-5
NCORES = 8
BL = B_ // NCORES          # local batch per core
TN = T_ * N_               # 7500
NCH = 15                   # 500-col chunks
CH = 500
MCH = 60                   # 125-col chunks
MC = 125

_CACHE = {}


def _build():
    import concourse.bacc as bacc
    import concourse.mybir as mybir
    from concourse import tile

    f32 = mybir.dt.float32
    bf16 = mybir.dt.bfloat16
    AF = mybir.ActivationFunctionType
    AX = mybir.AxisListType

    nc = bacc.Bacc("TRN2", target_bir_lowering=False, debug=False,
                   num_devices=NCORES)

    x = nc.dram_tensor("x", [BL, CIN, TN], f32, kind="ExternalInput")
    ksymT = nc.dram_tensor("ksymT", [CIN, CIN], bf16, kind="ExternalInput")
    wst = nc.dram_tensor("wst", [COUT, COUT], bf16, kind="ExternalInput")
    ident = nc.dram_tensor("ident", [CIN, CIN], bf16, kind="ExternalInput")
    asum = nc.dram_tensor("asum", [N_, N_], f32, kind="ExternalInput")
    bns = nc.dram_tensor("bns", [COUT, 1], f32, kind="ExternalInput")
    bnb = nc.dram_tensor("bnb", [COUT, 1], f32, kind="ExternalInput")
    out = nc.dram_tensor("out", [BL, COUT, TN], f32, kind="ExternalOutput")

    with tile.TileContext(nc) as tc:
        with (
            tc.tile_pool(name="const", bufs=1) as cpool,
            tc.tile_pool(name="stk", bufs=2) as stkpool,
            tc.tile_pool(name="p16", bufs=2) as ppool,
            tc.tile_pool(name="xt16", bufs=2) as xtpool,
            tc.tile_pool(name="osb", bufs=4) as opool,
            tc.tile_pool(name="small", bufs=16) as spool,
            tc.tile_pool(name="bd", bufs=2) as bdpool,
            tc.tile_pool(name="pps", bufs=2, space="PSUM") as ppsum,
            tc.tile_pool(name="mps", bufs=2, space="PSUM") as mpsum,
            tc.tile_pool(name="xtp", bufs=1, space="PSUM") as xtpsum,
            tc.tile_pool(name="xsp", bufs=1, space="PSUM") as xspsum,
            tc.tile_pool(name="ops", bufs=2, space="PSUM") as opsum,
        ):
            ks_t = cpool.tile([CIN, CIN], bf16)
            nc.sync.dma_start(ks_t[:], ksymT[:])
            w_t = cpool.tile([COUT, COUT], bf16)
            nc.sync.dma_start(w_t[:], wst[:])
            id_t = cpool.tile([CIN, CIN], bf16)
            nc.sync.dma_start(id_t[:], ident[:])
            as_t = cpool.tile([N_, N_], f32)
            nc.sync.dma_start(as_t[:], asum[:])
            bns_t = cpool.tile([COUT, 1], f32)
            nc.sync.dma_start(bns_t[:], bns[:])
            bnb_t = cpool.tile([COUT, 1], f32)
            nc.sync.dma_start(bnb_t[:], bnb[:])

            for b in range(BL):
                stk = stkpool.tile([COUT, TN], bf16)
                # cast-DMA (SWDGE): f32 DRAM -> bf16 SBUF, two halves
                half = TN // 2
                nc.gpsimd.dma_start(stk[0:64, 0:half], x[b, :, 0:half])
                nc.gpsimd.dma_start(stk[0:64, half:TN], x[b, :, half:TN])

                # ---- P = ksymT.T @ X ----
                p16 = ppool.tile([CIN, TN], bf16)
                for j in range(NCH):
                    sl = slice(j * CH, (j + 1) * CH)
                    pps = ppsum.tile([CIN, CH], f32)
                    nc.tensor.matmul(pps[:], ks_t[:], stk[0:64, sl],
                                     start=True, stop=True)
                    nc.scalar.activation(p16[:, sl], pps[:], AF.Copy)

                # ---- M accumulation + X^T transposes ----
                mps = mpsum.tile([MC, MC], f32)
                xt16 = xtpool.tile([MC, MCH * CIN], bf16)
                for g in range(NCH):
                    xtp = xtpsum.tile([MC, 4 * CIN], bf16)
                    for q in range(4):
                        ci = 4 * g + q
                        sl = slice(ci * MC, (ci + 1) * MC)
                        nc.tensor.matmul(mps[:], stk[0:64, sl], p16[:, sl],
                                         start=(ci == 0), stop=(ci == MCH - 1))
                        nc.tensor.transpose(xtp[:, q * CIN:(q + 1) * CIN],
                                            stk[0:64, sl], id_t[:])
                    nc.vector.tensor_copy(
                        xt16[:, g * 4 * CIN:(g + 1) * 4 * CIN], xtp[:])

                # ---- M diag-block reduce + softmax + adj ----
                # Engines can't address partition bases that aren't 0 mod 32,
                # so realign the 5 diagonal (25,25) blocks via SBUF->SBUF DMA.
                msb = spool.tile([MC, MC], f32, tag="msb")
                nc.vector.tensor_copy(msb[:], mps[:])
                mdiag = spool.tile([N_, 5 * N_], f32, tag="mdiag")
                for t in range(5):
                    nc.sync.dma_start(mdiag[:, t * 25:(t + 1) * 25],
                                      msb[t * 25:(t + 1) * 25, t * 25:(t + 1) * 25])
                m01 = spool.tile([N_, N_], f32, tag="sm")
                nc.vector.tensor_add(m01[:], mdiag[:, 0:25], mdiag[:, 25:50])
                m23 = spool.tile([N_, N_], f32, tag="sm")
                nc.vector.tensor_add(m23[:], mdiag[:, 50:75], mdiag[:, 75:100])
                m03 = spool.tile([N_, N_], f32, tag="sm")
                nc.vector.tensor_add(m03[:], m01[:], m23[:])
                msum = spool.tile([N_, N_], f32, tag="sm")
                nc.vector.tensor_add(msum[:], m03[:], mdiag[:, 100:125])

                negmax = spool.tile([N_, 1], f32, tag="sv")
                nc.vector.reduce_max(negmax[:], msum[:], axis=AX.X, negate=True)
                expm = spool.tile([N_, N_], f32, tag="sm")
                ssum = spool.tile([N_, 1], f32, tag="sv")
                nc.scalar.activation(expm[:], msum[:], AF.Exp,
                                     bias=negmax[:], accum_out=ssum[:])
                rs = spool.tile([N_, 1], f32, tag="sv")
                nc.vector.reciprocal(rs[:], ssum[:])
                rs3 = spool.tile([N_, 1], f32, tag="sv")
                nc.scalar.mul(rs3[:], rs[:], float(KV))
                adjf = spool.tile([N_, N_], f32, tag="sm")
                nc.vector.tensor_scalar_mul(adjf[:], expm[:], rs3[:])
                adj16 = spool.tile([N_, N_], bf16, tag="sm16")
                nc.vector.tensor_add(adj16[:], adjf[:], as_t[:])

                bd = bdpool.tile([MC, MC], bf16)
                nc.gpsimd.memset(bd[:], 0.0)
                for t in range(5):
                    nc.sync.dma_start(
                        bd[t * 25:(t + 1) * 25, t * 25:(t + 1) * 25], adj16[:])

                # ---- x_sum = blockdiag-adj applied to X -> stk rows 0:64 ----
                for g in range(NCH):
                    xsp = xspsum.tile([CIN, CH], f32)
                    for q in range(4):
                        ci = 4 * g + q
                        nc.tensor.matmul(xsp[:, q * MC:(q + 1) * MC],
                                         xt16[:, ci * CIN:(ci + 1) * CIN],
                                         bd[:], start=True, stop=True)
                    sl = slice(g * CH, (g + 1) * CH)
                    nc.scalar.activation(stk[64:128, sl], xsp[:], AF.Copy)

                # ---- out = [w1|wr] @ stk, BN+ReLU fused ----
                for j in range(NCH):
                    sl = slice(j * CH, (j + 1) * CH)
                    ops = opsum.tile([COUT, CH], f32)
                    nc.tensor.matmul(ops[:], w_t[:], stk[:, sl],
                                     start=True, stop=True)
                    osb = opool.tile([COUT, CH], f32)
                    nc.scalar.activation(osb[:], ops[:], AF.Relu,
                                         bias=bnb_t[:], scale=bns_t[:])
                    nc.sync.dma_start(out[b, :, sl], osb[:])
    nc.finalize()
    return nc


def kernel(**inputs):
    x = np.ascontiguousarray(inputs["x"], dtype=np.float32)
    theta_w = inputs["theta_w"]
    phi_w = inputs["phi_w"]
    A, Bp = inputs["A"], inputs["Bparam"]
    w1, wr = inputs["w1"], inputs["wr"]
    b1, br = inputs["b1"], inputs["br"]
    gamma, beta = inputs["gamma"], inputs["beta"]
    rmean, rvar = inputs["rmean"], inputs["rvar"]

    bf = ml_dtypes.bfloat16
    ksymT = np.ascontiguousarray(phi_w.T @ theta_w).astype(bf)
    wst = np.ascontiguousarray(
        np.concatenate([wr.T, w1.T], axis=0)).astype(bf)
    ident = np.eye(CIN, dtype=np.float32).astype(bf)
    asum = np.ascontiguousarray((A + Bp).sum(0), dtype=np.float32)
    bnscale = (gamma / np.sqrt(rvar + EPS)).astype(np.float32)
    bnbias = ((b1 + br - rmean) * bnscale + beta).astype(np.float32)

    if "nc" not in _CACHE:
        _CACHE["nc"] = _build()
    nc = _CACHE["nc"]

    shared = {
        "ksymT": ksymT, "wst": wst, "ident": ident, "asum": asum,
        "bns": np.ascontiguousarray(bnscale[:, None]),
        "bnb": np.ascontiguousarray(bnbias[:, None]),
    }
    in_maps = []
    for i in range(NCORES):
        xi = np.ascontiguousarray(
            x[i * BL:(i + 1) * BL].reshape(BL, CIN, TN))
        in_maps.append({"x": xi, **shared})

    from concourse.bass_utils import run_bass_kernel_spmd
    res = run_bass_kernel_spmd(nc, in_maps, core_ids=list(range(NCORES)))
    outs = [np.asarray(r["out"], dtype=np.float32).reshape(BL, COUT, T_, N_)
            for r in res.results]
    return np.concatenate(outs, axis=0)
